# revision 9
# baseline (speedup 1.0000x reference)
"""GAT 2-layer kernel for trn2, 8 NeuronCores, dst-sharded graph parallel.

Self-contained: host-side sharding/CSR build (numpy) + Bass/Tile device kernel.
"""
import os
import sys

sys.path.insert(0, "/opt/trn_rl_repo")

import inspect
import textwrap

import numpy as np

import concourse.bacc as bacc
import concourse.bass as bass
import concourse.mybir as mybir
import concourse.tile as tile
from concourse import library_config
from concourse.masks import make_identity
from concourse.bass_utils import run_bass_kernel_spmd

# ---- problem constants (hardcoded per harness contract) ----
N = 100000
IN_DIM = 512
HID = 16
NCLS = 16
NEG = 0.2
NC = 8                    # cores
NLOC = 12500              # nodes per core
NT = 98                   # tiles of 128 positions per core (98*128 = 12544)
NPOS = NT * 128           # positions incl. dummies
SLICE = 12800             # table slice rows per core (1 sentinel + 12544 + pad)
TROWS = SLICE * NC        # 102400 table rows
SLAB = 32768              # int16 index window
NSLAB = 4
GT = 7                    # tiles per gather group
NG = NT // GT             # 14 groups
ROWW = 17                 # compact table row width (h16 | as)
PADW = 64                 # padded table row width (256B stride)
NEGBIG = -1.0e30
NSQ = int(os.environ.get("GAT_NSQ", "4"))
STAGES = int(os.environ.get("GAT_STAGES", "99"))

_patched = False


def _patch_dma_gather():
    global _patched
    if _patched:
        return
    src = inspect.getsource(bass.BassGpSimd.dma_gather)
    p = src.replace(
        "elem_size_bytes > 0 and elem_size_bytes % 256 == 0", "elem_size_bytes > 0"
    )
    assert p != src, "dma_gather elem assert not found"
    ns = dict(bass.__dict__)
    exec(compile(textwrap.dedent(p), "<patched_dma_gather>", "exec"), ns)
    bass.BassGpSimd.dma_gather = ns["dma_gather"]
    _patched = True


def _ap(t_ap, dims, off=0):
    """Build an AP view with explicit free dims on top of a tile's partition dim."""
    return bass.AP(t_ap.tensor, t_ap.offset + off,
                   [list(t_ap.ap[0])] + [list(d) for d in dims])


# slab sentinels (global table rows; each inside its 32768-row window)
SENT = [0, 3 * SLICE, 6 * SLICE, 7 * SLICE + SLICE - 1]  # 0,38400,76800,102399
for _s in range(NSLAB):
    assert _s * SLAB <= SENT[_s] < min((_s + 1) * SLAB, TROWS)


def _host_prep(x, edge_index):
    """Shard + CSR build. Returns per-core upload dicts + unshard info."""
    src = np.concatenate([edge_index[0], np.arange(N, dtype=np.int64)])
    dst = np.concatenate([edge_index[1], np.arange(N, dtype=np.int64)])
    src = src.astype(np.int64)
    dst = dst.astype(np.int64)
    order = np.argsort(dst, kind="stable")
    dsts = dst[order]
    srcs = src[order]
    core_edge_bounds = np.searchsorted(dsts, np.arange(NC + 1) * NLOC)
    deg = np.bincount(dst, minlength=N)

    # per-core degree-sorted position assignment
    perms = []          # perms[c][q] = original local node id (q < 12500)
    table_row = np.empty(N, dtype=np.int64)   # node -> global table row
    for c in range(NC):
        dloc = deg[c * NLOC:(c + 1) * NLOC]
        perm = np.argsort(-dloc, kind="stable")   # positions 0..12499
        perms.append(perm)
        q = np.arange(NLOC)
        t = q // 128
        p = q % 128
        rowloc = 1 + p * NT + t
        table_row[c * NLOC + perm] = c * SLICE + rowloc
    inputs = []
    for c in range(NC):
        e0, e1 = core_edge_bounds[c], core_edge_bounds[c + 1]
        es = srcs[e0:e1]
        ed = dsts[e0:e1] - c * NLOC
        perm = perms[c]
        qinv = np.empty(NLOC, dtype=np.int64)
        qinv[perm] = np.arange(NLOC)
        eq = qinv[ed]                       # dst position 0..12499
        erow = table_row[es]                # src global table row
        eslab = erow // SLAB
        et = eq // 128
        ep = eq % 128
        eg = et // GT
        etl = et % GT

        # per (position, slab) occurrence index j
        key = (eq * NSLAB + eslab).astype(np.int64)
        o2 = np.argsort(key, kind="stable")
        ks = key[o2]
        newrun = np.empty(len(ks), dtype=bool)
        if len(ks):
            newrun[0] = True
            newrun[1:] = ks[1:] != ks[:-1]
        runid = np.cumsum(newrun) - 1
        runstart = np.flatnonzero(newrun)
        j_sorted = np.arange(len(ks)) - runstart[runid]
        ej = np.empty(len(ks), dtype=np.int64)
        ej[o2] = j_sorted

        cnt = np.zeros((NPOS, NSLAB), dtype=np.int64)
        np.add.at(cnt, (np.minimum(eq, NPOS - 1), eslab), 1)

        # per (group, slab) width
        Wgs = np.zeros((NG, NSLAB), dtype=np.int64)
        for g in range(NG):
            seg = cnt[g * GT * 128:(g + 1) * GT * 128]
            Wgs[g] = seg.max(axis=0)
        Wgs = np.maximum(Wgs, 1)

        # assemble idx arrays per (g, s): flat k = ((tl*W + j)*128 + p)
        call_meta = []   # (g, s, W, s16_offset)
        idx_chunks = []
        off16 = 0
        for g in range(NG):
            for s in range(NSLAB):
                W = int(Wgs[g, s])
                slots = GT * 128 * W
                flat = np.full(slots, SENT[s] - s * SLAB, dtype=np.int64)
                m = (eg == g) & (eslab == s)
                k = (etl[m] * W + ej[m]) * 128 + ep[m]
                flat[k] = erow[m] - s * SLAB
                assert flat.max() < SLAB and flat.min() >= 0
                s16 = slots // 16
                wrapped = flat.reshape(s16, 16).T.astype(np.int16)   # [16, s16]
                idx_chunks.append(np.tile(wrapped, (8, 1)))          # [128, s16]
                call_meta.append((g, s, W, off16))
                off16 += s16
        idx_all = np.concatenate(idx_chunks, axis=1)  # [128, off16]

        # x in position order, tiled [NT, 4, 128, 128]
        xs = np.zeros((NPOS, IN_DIM), dtype=np.float32)
        xs[:NLOC] = x[c * NLOC + perm]
        x_tiled = np.ascontiguousarray(
            xs.reshape(NT, 128, 4, 128).transpose(0, 2, 3, 1)
        )
        inputs.append(dict(x_tiled=x_tiled, idx_all=idx_all,
                           call_meta=call_meta, perm=perm))
    return inputs, core_edge_bounds


def _build_program(call_meta, idx_cols):
    nc = bacc.Bacc("TRN2", target_bir_lowering=False, debug=False,
                   num_devices=NC, num_swdge_queues=NSQ)
    f32 = mybir.dt.float32
    i16 = mybir.dt.int16

    xt = nc.declare_dram_parameter("x_tiled", [NT, 4, 128, 128], f32, isOutput=False)
    idx_all = nc.declare_dram_parameter("idx_all", [128, idx_cols], i16, isOutput=False)
    wf1 = nc.declare_dram_parameter("wf1", [4, 128, 18], f32, isOutput=False)
    wf2 = nc.declare_dram_parameter("wf2", [16, 18], f32, isOutput=False)
    wo = nc.declare_dram_parameter("wo", [16, 16], f32, isOutput=False)
    bvec = nc.declare_dram_parameter("bvec", [1, 48], f32, isOutput=False)  # b1,b2,bout
    ident_d = nc.declare_dram_parameter("ident", [128, 128], f32, isOutput=False)
    out_d = nc.declare_dram_parameter("out", [NPOS, 16], f32, isOutput=True)

    slice1 = nc.dram_tensor("slice1", [SLICE, ROWW], f32)
    slice2 = nc.dram_tensor("slice2", [SLICE, ROWW], f32)
    tab1c = nc.dram_tensor("tab1c", [TROWS, ROWW], f32, addr_space="Shared")
    tab2c = nc.dram_tensor("tab2c", [TROWS, ROWW], f32, addr_space="Shared")
    tab1p = nc.dram_tensor("tab1p", [TROWS, PADW], f32)
    tab2p = nc.dram_tensor("tab2p", [TROWS, PADW], f32)

    RPP = TROWS // 128          # 800 compact rows per partition
    CH = 8
    RPC = RPP // CH             # 100 rows per expansion chunk

    with tile.TileContext(nc) as tc:
        with (
            tc.tile_pool(name="const", bufs=1) as cpool,
            tc.tile_pool(name="work", bufs=2) as pool,
            tc.tile_pool(name="psum", bufs=2, space="PSUM") as pp,
        ):
            nc.gpsimd.load_library(library_config.mlp)
            wf1_sb = cpool.tile([128, 4 * 18], f32)
            for k in range(4):
                nc.sync.dma_start(out=wf1_sb[:, k * 18:(k + 1) * 18], in_=wf1[k])
            wf2_sb = cpool.tile([16, 18], f32)
            nc.sync.dma_start(out=wf2_sb[:], in_=wf2[:])
            wo_sb = cpool.tile([16, 16], f32)
            nc.sync.dma_start(out=wo_sb[:], in_=wo[:])
            b_sb = cpool.tile([1, 48], f32)
            nc.sync.dma_start(out=b_sb[:], in_=bvec[:])
            ones1 = cpool.tile([1, 128], f32)
            nc.vector.memset(ones1[:], 1.0)
            bfull = cpool.tile([128, 48], f32)
            for bi in range(3):
                psb = pp.tile([128, 16], f32, tag="ps_b")
                nc.tensor.matmul(out=psb[:], lhsT=ones1[:], rhs=b_sb[:, bi * 16:(bi + 1) * 16],
                                 start=True, stop=True)
                nc.vector.tensor_copy(out=bfull[:, bi * 16:(bi + 1) * 16], in_=psb[:])
            sent_sb = cpool.tile([128, ROWW], f32)
            nc.vector.memset(sent_sb[:], 0.0)
            nc.vector.memset(sent_sb[:, 16:17], NEGBIG)
            ident = cpool.tile([128, 128], f32)
            nc.sync.dma_start(out=ident[:], in_=ident_d[:])

            ad1 = cpool.tile([128, NT], f32)
            ad2 = cpool.tile([128, NT], f32)
            h2in = cpool.tile([128, NT * 16], f32)
            o2in = cpool.tile([128, NT * 16], f32)
            o3 = cpool.tile([128, NT * 16], f32)
            stage = cpool.tile([128, NT * ROWW], f32)

            def write_slice(slice_d):
                nc.sync.dma_start(
                    out=bass.AP(slice_d[:].tensor, ROWW,
                                [[NT * ROWW, 128], [1, NT * ROWW]]),
                    in_=stage[:],
                )
                nc.sync.dma_start(out=slice_d[0:1, :], in_=sent_sb[:1, :])
                nc.sync.dma_start(out=slice_d[1 + NPOS:1 + NPOS + 128, :], in_=sent_sb[:])
                nc.sync.dma_start(
                    out=slice_d[1 + NPOS + 128:SLICE, :],
                    in_=sent_sb[:SLICE - NPOS - 129, :],
                )

            def allgather(slice_d, tabc):
                nc.gpsimd.collective_compute(
                    "AllGather",
                    mybir.AluOpType.bypass,
                    replica_groups=[list(range(NC))],
                    ins=[slice_d[:]],
                    outs=[tabc[:]],
                )

            def expand(tabc, tabp):
                for ci in range(CH):
                    comp = pool.tile([128, RPC * ROWW], f32, tag="comp")
                    nc.sync.dma_start(
                        out=comp[:],
                        in_=bass.AP(tabc[:].tensor, ci * RPC * ROWW,
                                    [[RPP * ROWW, 128], [1, RPC * ROWW]]),
                    )
                    pad = pool.tile([128, RPC * PADW], f32, tag="pad")
                    nc.vector.tensor_copy(
                        out=_ap(pad[:], [[PADW, RPC], [1, ROWW]]),
                        in_=_ap(comp[:], [[ROWW, RPC], [1, ROWW]]),
                    )
                    nc.sync.dma_start(
                        out=bass.AP(tabp[:].tensor, ci * RPC * PADW,
                                    [[RPP * PADW, 128], [1, RPC * PADW]]),
                        in_=pad[:],
                    )

            def node_stage1():
                for t in range(NT):
                    ps = pp.tile([128, 18], f32, tag="ps_n")
                    for k in range(4):
                        xk = pool.tile([128, 128], f32, tag="xk")
                        nc.sync.dma_start(out=xk[:], in_=xt[t, k])
                        nc.tensor.matmul(
                            out=ps[:], lhsT=xk[:], rhs=wf1_sb[:, k * 18:(k + 1) * 18],
                            start=(k == 0), stop=(k == 3),
                        )
                    nc.scalar.copy(out=stage[:, t * ROWW:(t + 1) * ROWW], in_=ps[:, 0:17])
                    nc.vector.tensor_copy(out=ad1[:, t:t + 1], in_=ps[:, 17:18])

            def edge_stage(tabp, adcol, dst_sb, bias_row, meta_off):
                # returns per-tile output written into dst_sb cols (after relu)
                gbuf = {}
                for (g, s, W, off16) in call_meta:
                    gt_ = pool.tile([128, GT * W * ROWW], f32, tag="g%d" % s)
                    for tl0 in range(0, GT, 2):
                        ntl = min(2, GT - tl0)
                        nslots = ntl * 128 * W
                        it = pool.tile([128, GT * 128 * W // 16], i16,
                                       tag="it%d" % (s % 2))
                        nc.sync.dma_start(
                            out=it[:, :nslots // 16],
                            in_=idx_all[:, off16 + tl0 * 8 * W:
                                        off16 + tl0 * 8 * W + nslots // 16],
                        )
                        nc.gpsimd.dma_gather(
                            _ap(gt_[:], [[1, ntl * W], [1, ROWW]],
                                off=tl0 * W * ROWW).rearrange("p (j) w -> p j w")
                            if False else
                            bass.AP(gt_[:].tensor, gt_[:].offset + tl0 * W * ROWW,
                                    [list(gt_[:].ap[0]), [ROWW, ntl * W], [1, ROWW]]),
                            tabp[s * SLAB:min((s + 1) * SLAB, TROWS), :ROWW],
                            it[:, :nslots // 16],
                            nslots,
                            nslots,
                            ROWW,
                            elem_step=PADW,
                            single_packet=False,
                            queue_num=s % NSQ,
                        )
                    gbuf[s] = (gt_, W)
                    if s == NSLAB - 1:
                        for tl in range(GT):
                            t = g * GT + tl
                            Wt = sum(gbuf[ss][1] for ss in range(NSLAB))
                            L = pool.tile([128, Wt], f32, tag="L")
                            wv = pool.tile([128, Wt], f32, tag="wv")
                            wh = pool.tile([128, Wt * 16], f32, tag="wh")
                            off = 0
                            for ss in range(NSLAB):
                                gts, Ws = gbuf[ss]
                                nc.scalar.activation(
                                    out=L[:, off:off + Ws],
                                    in_=_ap(gts[:], [[ROWW, Ws]],
                                            off=tl * Ws * ROWW + 16),
                                    func=mybir.ActivationFunctionType.Identity,
                                    bias=adcol[:, t:t + 1],
                                )
                                off += Ws
                            nc.vector.scalar_tensor_tensor(
                                out=L[:], in0=L[:], scalar=NEG, in1=L[:],
                                op0=mybir.AluOpType.mult,
                                op1=mybir.AluOpType.max,
                            )
                            nc.vector.tensor_scalar_min(L[:], L[:], 60.0)
                            den = pool.tile([128, 1], f32, tag="den")
                            nc.scalar.activation(
                                out=wv[:], in_=L[:],
                                func=mybir.ActivationFunctionType.Exp,
                                accum_out=den[:],
                            )
                            off = 0
                            for ss in range(NSLAB):
                                gts, Ws = gbuf[ss]
                                nc.vector.tensor_tensor(
                                    out=_ap(wh[:], [[16, Ws], [1, 16]], off=off * 16),
                                    in0=_ap(gts[:], [[ROWW, Ws], [1, 16]],
                                            off=tl * Ws * ROWW),
                                    in1=_ap(wv[:], [[1, Ws], [0, 16]], off=off),
                                    op=mybir.AluOpType.mult,
                                )
                                off += Ws
                            num = pool.tile([128, 16], f32, tag="num")
                            nc.vector.tensor_reduce(
                                out=num[:],
                                in_=_ap(wh[:], [[1, 16], [16, Wt]]),
                                axis=mybir.AxisListType.X,
                                op=mybir.AluOpType.add,
                            )
                            rden = pool.tile([128, 1], f32, tag="rden")
                            nc.vector.tensor_scalar_add(den[:], den[:], 1e-30)
                            nc.vector.reciprocal(rden[:], den[:])
                            ot = pool.tile([128, 16], f32, tag="ot")
                            nc.vector.scalar_tensor_tensor(
                                out=ot[:], in0=num[:], scalar=rden[:],
                                in1=bias_row,
                                op0=mybir.AluOpType.mult,
                                op1=mybir.AluOpType.add,
                            )
                            nc.scalar.activation(
                                out=dst_sb[:, t * 16:(t + 1) * 16], in_=ot[:],
                                func=mybir.ActivationFunctionType.Relu,
                            )

            def node_stage2(src_sb, wmat, wcols, dst_stage, adcol):
                for t in range(NT):
                    tp = pp.tile([16, 128], f32, tag="ps_t")
                    nc.tensor.transpose(
                        out=tp[:], in_=src_sb[:, t * 16:(t + 1) * 16], identity=ident[:]
                    )
                    o1t = pool.tile([16, 128], f32, tag="o1t")
                    nc.vector.tensor_copy(out=o1t[:], in_=tp[:])
                    ps = pp.tile([128, wcols], f32, tag="ps_m")
                    nc.tensor.matmul(out=ps[:], lhsT=o1t[:], rhs=wmat[:, :wcols],
                                     start=True, stop=True)
                    if dst_stage is not None:
                        nc.scalar.copy(
                            out=dst_stage[:, t * ROWW:(t + 1) * ROWW], in_=ps[:, 0:17]
                        )
                        nc.vector.tensor_copy(out=adcol[:, t:t + 1], in_=ps[:, 17:18])
                    else:
                        # head: ps = logits pre-bias [128,16]
                        lg = pool.tile([128, 16], f32, tag="lg")
                        nc.vector.tensor_tensor(
                            out=lg[:], in0=ps[:, 0:16],
                            in1=bfull[:, 32:48],
                            op=mybir.AluOpType.add,
                        )
                        mx = pool.tile([128, 1], f32, tag="mx")
                        nc.vector.tensor_reduce(
                            out=mx[:], in_=lg[:], axis=mybir.AxisListType.X,
                            op=mybir.AluOpType.max,
                        )
                        nm = pool.tile([128, 1], f32, tag="nm")
                        nc.vector.tensor_scalar_mul(nm[:], mx[:], -1.0)
                        ex = pool.tile([128, 16], f32, tag="ex")
                        rs = pool.tile([128, 1], f32, tag="rs")
                        nc.scalar.activation(
                            out=ex[:], in_=lg[:],
                            func=mybir.ActivationFunctionType.Exp,
                            bias=nm[:], accum_out=rs[:],
                        )
                        rr = pool.tile([128, 1], f32, tag="rr")
                        nc.vector.reciprocal(rr[:], rs[:])
                        nc.vector.tensor_scalar_mul(
                            _ap(o3[:], [[1, 16]], off=t * 16), ex[:], rr[:]
                        )

            # ---- pipeline ----
            node_stage1()
            write_slice(slice1)
            if STAGES >= 2:
                allgather(slice1, tab1c)
            if STAGES >= 3:
                expand(tab1c, tab1p)
            if STAGES >= 4:
                edge_stage(tab1p, ad1, h2in, bfull[:, 0:16], 0)
            if STAGES >= 5:
                node_stage2(h2in, wf2_sb, 18, stage, ad2)
                write_slice(slice2)
                allgather(slice2, tab2c)
                expand(tab2c, tab2p)
                edge_stage(tab2p, ad2, o2in, bfull[:, 16:32], 0)
                node_stage2(o2in, wo_sb, 16, None, None)
            if STAGES < 5:
                nc.vector.memset(o3[:], 0.5)
            nc.sync.dma_start(
                out=bass.AP(out_d[:].tensor, 0, [[NT * 16, 128], [1, NT * 16]]),
                in_=o3[:],
            )
    nc.compile()
    return nc


_prog_cache = {}


def kernel(**inputs):
    _patch_dma_gather()
    x = np.asarray(inputs["x"], dtype=np.float32)
    edge_index = np.asarray(inputs["edge_index"])
    W1 = np.asarray(inputs["W1"], dtype=np.float32)
    a_src1 = np.asarray(inputs["a_src1"], dtype=np.float32)
    a_dst1 = np.asarray(inputs["a_dst1"], dtype=np.float32)
    b1 = np.asarray(inputs["b1"], dtype=np.float32)
    W2 = np.asarray(inputs["W2"], dtype=np.float32)
    a_src2 = np.asarray(inputs["a_src2"], dtype=np.float32)
    a_dst2 = np.asarray(inputs["a_dst2"], dtype=np.float32)
    b2 = np.asarray(inputs["b2"], dtype=np.float32)
    Wout = np.asarray(inputs["Wout"], dtype=np.float32)
    bout = np.asarray(inputs["bout"], dtype=np.float32)

    prep, _ = _host_prep(x, edge_index)
    wf1 = np.concatenate(
        [W1, (W1 @ a_src1)[:, None], (W1 @ a_dst1)[:, None]], axis=1
    ).astype(np.float32)                                  # [512, 18]
    wf1_t = np.ascontiguousarray(wf1.reshape(4, 128, 18))
    wf2 = np.concatenate(
        [W2, (W2 @ a_src2)[:, None], (W2 @ a_dst2)[:, None]], axis=1
    ).astype(np.float32)                                  # [16, 18]
    bvec = np.concatenate([b1, b2, bout]).astype(np.float32)[None, :]

    # program structure depends on call widths (same for all cores? NO - per core).
    # Build per-core programs is not possible in SPMD; use core 0's meta? Widths
    # differ per core -> pad to the max across cores per (g, s).
    metas = [p["call_meta"] for p in prep]
    merged = []
    for i in range(len(metas[0])):
        g, s, _, _ = metas[0][i]
        W = max(m[i][2] for m in metas)
        merged.append((g, s, W, 0))
    # rebuild idx arrays at merged widths
    off16 = 0
    final_meta = []
    for (g, s, W, _) in merged:
        final_meta.append((g, s, W, off16))
        off16 += GT * 128 * W // 16
    idx_cols = off16

    for c in range(NC):
        idx_new = np.empty((128, idx_cols), dtype=np.int16)
        for i, (g, s, W, off) in enumerate(final_meta):
            _, _, Wc, offc = prep[c]["call_meta"][i]
            s16c = GT * 128 * Wc // 16
            s16 = GT * 128 * W // 16
            chunk = prep[c]["idx_all"][:, offc:offc + s16c]
            # re-pad from width Wc to W: flat k = ((tl*W + j)*128 + p)
            flat_c = np.empty(GT * 128 * Wc, dtype=np.int16)
            wr = chunk[:16, :]                       # [16, s16c]
            flat_c = wr.T.reshape(-1)                # k = s*16+p order
            a3 = flat_c.reshape(GT, Wc, 128)
            a3n = np.full((GT, W, 128), SENT[s] - s * SLAB, dtype=np.int16)
            a3n[:, :Wc, :] = a3
            flat_n = a3n.reshape(-1)
            wrapped = flat_n.reshape(s16, 16).T
            idx_new[:, off:off + s16] = np.tile(wrapped, (8, 1))
        prep[c]["idx_all"] = idx_new

    key = (STAGES,) + tuple((m[0], m[1], m[2]) for m in final_meta)
    if key not in _prog_cache:
        _prog_cache[key] = _build_program(final_meta, idx_cols)
    nc = _prog_cache[key]

    in_maps = []
    for c in range(NC):
        in_maps.append({
            "x_tiled": prep[c]["x_tiled"],
            "idx_all": prep[c]["idx_all"],
            "wf1": wf1_t, "wf2": wf2, "wo": Wout.astype(np.float32),
            "bvec": bvec, "ident": np.eye(128, dtype=np.float32),
        })
    res = run_bass_kernel_spmd(nc, in_maps, list(range(NC)))

    out = np.empty((N, NCLS), dtype=np.float32)
    q = np.arange(NPOS)
    r_of_q = (q % 128) * NT + q // 128
    for c in range(NC):
        ob = res.results[c]["out"]          # [NPOS, 16] rows r = p*NT+t
        byq = ob[r_of_q]                    # position-ordered
        out[c * NLOC + prep[c]["perm"]] = byq[:NLOC]
    return out


# revision 10
# speedup vs baseline: 1.3128x; 1.3128x over previous
"""GAT 2-layer kernel for trn2, 8 NeuronCores, dst-sharded graph parallel.

Self-contained: host-side sharding/CSR build (numpy) + Bass/Tile device kernel.
"""
import os
import sys

sys.path.insert(0, "/opt/trn_rl_repo")

import inspect
import textwrap

import numpy as np

import concourse.bacc as bacc
import concourse.bass as bass
import concourse.mybir as mybir
import concourse.tile as tile
from concourse import library_config
from concourse.masks import make_identity
from concourse.bass_utils import run_bass_kernel_spmd

# ---- problem constants (hardcoded per harness contract) ----
N = 100000
IN_DIM = 512
HID = 16
NCLS = 16
NEG = 0.2
NC = 8                    # cores
NLOC = 12500              # nodes per core
NT = 98                   # tiles of 128 positions per core (98*128 = 12544)
NPOS = NT * 128           # positions incl. dummies
SLICE = 12800             # table slice rows per core (1 sentinel + 12544 + pad)
TROWS = SLICE * NC        # 102400 table rows
SLAB = 32768              # int16 index window
NSLAB = 4
GT = 7                    # tiles per gather group
NG = NT // GT             # 14 groups
ROWW = 17                 # compact table row width (h16 | as)
PADW = 64                 # padded table row width (256B stride)
NEGBIG = -1.0e30
NSQ = int(os.environ.get("GAT_NSQ", "4"))
STAGES = int(os.environ.get("GAT_STAGES", "99"))

_patched = False


def _patch_dma_gather():
    global _patched
    if _patched:
        return
    src = inspect.getsource(bass.BassGpSimd.dma_gather)
    p = src.replace(
        "elem_size_bytes > 0 and elem_size_bytes % 256 == 0", "elem_size_bytes > 0"
    )
    assert p != src, "dma_gather elem assert not found"
    ns = dict(bass.__dict__)
    exec(compile(textwrap.dedent(p), "<patched_dma_gather>", "exec"), ns)
    bass.BassGpSimd.dma_gather = ns["dma_gather"]
    _patched = True


def _ap(t_ap, dims, off=0):
    """Build an AP view with explicit free dims on top of a tile's partition dim."""
    return bass.AP(t_ap.tensor, t_ap.offset + off,
                   [list(t_ap.ap[0])] + [list(d) for d in dims])


# slab sentinels (global table rows; each inside its 32768-row window)
SENT = [0, 3 * SLICE, 6 * SLICE, 7 * SLICE + SLICE - 1]  # 0,38400,76800,102399
for _s in range(NSLAB):
    assert _s * SLAB <= SENT[_s] < min((_s + 1) * SLAB, TROWS)


def _host_prep(x, edge_index):
    """Shard + CSR build. Returns per-core upload dicts + unshard info."""
    src = np.concatenate([edge_index[0], np.arange(N, dtype=np.int64)])
    dst = np.concatenate([edge_index[1], np.arange(N, dtype=np.int64)])
    src = src.astype(np.int64)
    dst = dst.astype(np.int64)
    order = np.argsort(dst, kind="stable")
    dsts = dst[order]
    srcs = src[order]
    core_edge_bounds = np.searchsorted(dsts, np.arange(NC + 1) * NLOC)
    deg = np.bincount(dst, minlength=N)

    # per-core degree-sorted position assignment
    perms = []          # perms[c][q] = original local node id (q < 12500)
    table_row = np.empty(N, dtype=np.int64)   # node -> global table row
    for c in range(NC):
        dloc = deg[c * NLOC:(c + 1) * NLOC]
        perm = np.argsort(-dloc, kind="stable")   # positions 0..12499
        perms.append(perm)
        q = np.arange(NLOC)
        t = q // 128
        p = q % 128
        rowloc = 1 + p * NT + t
        table_row[c * NLOC + perm] = c * SLICE + rowloc
    inputs = []
    for c in range(NC):
        e0, e1 = core_edge_bounds[c], core_edge_bounds[c + 1]
        es = srcs[e0:e1]
        ed = dsts[e0:e1] - c * NLOC
        perm = perms[c]
        qinv = np.empty(NLOC, dtype=np.int64)
        qinv[perm] = np.arange(NLOC)
        eq = qinv[ed]                       # dst position 0..12499
        erow = table_row[es]                # src global table row
        eslab = erow // SLAB
        et = eq // 128
        ep = eq % 128
        eg = et // GT
        etl = et % GT

        # per (position, slab) occurrence index j
        key = (eq * NSLAB + eslab).astype(np.int64)
        o2 = np.argsort(key, kind="stable")
        ks = key[o2]
        newrun = np.empty(len(ks), dtype=bool)
        if len(ks):
            newrun[0] = True
            newrun[1:] = ks[1:] != ks[:-1]
        runid = np.cumsum(newrun) - 1
        runstart = np.flatnonzero(newrun)
        j_sorted = np.arange(len(ks)) - runstart[runid]
        ej = np.empty(len(ks), dtype=np.int64)
        ej[o2] = j_sorted

        cnt = np.zeros((NPOS, NSLAB), dtype=np.int64)
        np.add.at(cnt, (np.minimum(eq, NPOS - 1), eslab), 1)

        # per (group, slab) width
        Wgs = np.zeros((NG, NSLAB), dtype=np.int64)
        for g in range(NG):
            seg = cnt[g * GT * 128:(g + 1) * GT * 128]
            Wgs[g] = seg.max(axis=0)
        Wgs = np.maximum(Wgs, 1)

        # assemble idx arrays per (g, s): flat k = ((tl*W + j)*128 + p)
        call_meta = []   # (g, s, W, s16_offset)
        idx_chunks = []
        off16 = 0
        for g in range(NG):
            for s in range(NSLAB):
                W = int(Wgs[g, s])
                slots = GT * 128 * W
                flat = np.full(slots, SENT[s] - s * SLAB, dtype=np.int64)
                m = (eg == g) & (eslab == s)
                k = (etl[m] * W + ej[m]) * 128 + ep[m]
                flat[k] = erow[m] - s * SLAB
                assert flat.max() < SLAB and flat.min() >= 0
                s16 = slots // 16
                wrapped = flat.reshape(s16, 16).T.astype(np.int16)   # [16, s16]
                idx_chunks.append(np.tile(wrapped, (8, 1)))          # [128, s16]
                call_meta.append((g, s, W, off16))
                off16 += s16
        idx_all = np.concatenate(idx_chunks, axis=1)  # [128, off16]

        # x in position order, tiled [NT, 4, 128, 128]
        xs = np.zeros((NPOS, IN_DIM), dtype=np.float32)
        xs[:NLOC] = x[c * NLOC + perm]
        x_tiled = np.ascontiguousarray(
            xs.reshape(NT, 128, 4, 128).transpose(0, 2, 3, 1)
        )
        inputs.append(dict(x_tiled=x_tiled, idx_all=idx_all,
                           call_meta=call_meta, perm=perm))
    return inputs, core_edge_bounds


def _build_program(call_meta, idx_cols):
    nc = bacc.Bacc("TRN2", target_bir_lowering=False, debug=False,
                   num_devices=NC, num_swdge_queues=NSQ)
    f32 = mybir.dt.float32
    i16 = mybir.dt.int16

    xt = nc.declare_dram_parameter("x_tiled", [NT, 4, 128, 128], f32, isOutput=False)
    idx_all = nc.declare_dram_parameter("idx_all", [128, idx_cols], i16, isOutput=False)
    wf1 = nc.declare_dram_parameter("wf1", [4, 128, 18], f32, isOutput=False)
    wf2 = nc.declare_dram_parameter("wf2", [16, 18], f32, isOutput=False)
    wo = nc.declare_dram_parameter("wo", [16, 16], f32, isOutput=False)
    bvec = nc.declare_dram_parameter("bvec", [1, 48], f32, isOutput=False)  # b1,b2,bout
    ident_d = nc.declare_dram_parameter("ident", [128, 128], f32, isOutput=False)
    out_d = nc.declare_dram_parameter("out", [NPOS, 16], f32, isOutput=True)

    slice1 = nc.dram_tensor("slice1", [SLICE, ROWW], f32)
    slice2 = nc.dram_tensor("slice2", [SLICE, ROWW], f32)
    tab1c = nc.dram_tensor("tab1c", [TROWS, ROWW], f32, addr_space="Shared")
    tab2c = nc.dram_tensor("tab2c", [TROWS, ROWW], f32, addr_space="Shared")
    tab1p = nc.dram_tensor("tab1p", [TROWS, PADW], f32)
    tab2p = nc.dram_tensor("tab2p", [TROWS, PADW], f32)

    RPP = TROWS // 128          # 800 compact rows per partition
    CH = 8
    RPC = RPP // CH             # 100 rows per expansion chunk

    with tile.TileContext(nc) as tc:
        with (
            tc.tile_pool(name="const", bufs=1) as cpool,
            tc.tile_pool(name="work", bufs=2) as pool,
            tc.tile_pool(name="psum", bufs=2, space="PSUM") as pp,
        ):
            nc.gpsimd.load_library(library_config.mlp)
            wf1_sb = cpool.tile([128, 4 * 18], f32)
            for k in range(4):
                nc.sync.dma_start(out=wf1_sb[:, k * 18:(k + 1) * 18], in_=wf1[k])
            wf2_sb = cpool.tile([16, 18], f32)
            nc.sync.dma_start(out=wf2_sb[:], in_=wf2[:])
            wo_sb = cpool.tile([16, 16], f32)
            nc.sync.dma_start(out=wo_sb[:], in_=wo[:])
            b_sb = cpool.tile([1, 48], f32)
            nc.sync.dma_start(out=b_sb[:], in_=bvec[:])
            ones1 = cpool.tile([1, 128], f32)
            nc.vector.memset(ones1[:], 1.0)
            bfull = cpool.tile([128, 48], f32)
            for bi in range(3):
                psb = pp.tile([128, 16], f32, tag="ps_b")
                nc.tensor.matmul(out=psb[:], lhsT=ones1[:], rhs=b_sb[:, bi * 16:(bi + 1) * 16],
                                 start=True, stop=True)
                nc.vector.tensor_copy(out=bfull[:, bi * 16:(bi + 1) * 16], in_=psb[:])
            sent_sb = cpool.tile([128, ROWW], f32)
            nc.vector.memset(sent_sb[:], 0.0)
            nc.vector.memset(sent_sb[:, 16:17], NEGBIG)
            ident = cpool.tile([128, 128], f32)
            nc.sync.dma_start(out=ident[:], in_=ident_d[:])

            ad1 = cpool.tile([128, NT], f32)
            ad2 = cpool.tile([128, NT], f32)
            h2in = cpool.tile([128, NT * 16], f32)
            o2in = cpool.tile([128, NT * 16], f32)
            o3 = cpool.tile([128, NT * 16], f32)
            stage = cpool.tile([128, NT * ROWW], f32)

            def write_slice(slice_d):
                nc.sync.dma_start(
                    out=bass.AP(slice_d[:].tensor, ROWW,
                                [[NT * ROWW, 128], [1, NT * ROWW]]),
                    in_=stage[:],
                )
                nc.sync.dma_start(out=slice_d[0:1, :], in_=sent_sb[:1, :])
                nc.sync.dma_start(out=slice_d[1 + NPOS:1 + NPOS + 128, :], in_=sent_sb[:])
                nc.sync.dma_start(
                    out=slice_d[1 + NPOS + 128:SLICE, :],
                    in_=sent_sb[:SLICE - NPOS - 129, :],
                )

            def allgather(slice_d, tabc):
                nc.gpsimd.collective_compute(
                    "AllGather",
                    mybir.AluOpType.bypass,
                    replica_groups=[list(range(NC))],
                    ins=[slice_d[:]],
                    outs=[tabc[:]],
                )

            def expand(tabc, tabp):
                for ci in range(CH):
                    comp = pool.tile([128, RPC * ROWW], f32, tag="comp")
                    nc.sync.dma_start(
                        out=comp[:],
                        in_=bass.AP(tabc[:].tensor, ci * RPC * ROWW,
                                    [[RPP * ROWW, 128], [1, RPC * ROWW]]),
                    )
                    pad = pool.tile([128, RPC * PADW], f32, tag="pad")
                    nc.vector.tensor_copy(
                        out=_ap(pad[:], [[PADW, RPC], [1, ROWW]]),
                        in_=_ap(comp[:], [[ROWW, RPC], [1, ROWW]]),
                    )
                    nc.sync.dma_start(
                        out=bass.AP(tabp[:].tensor, ci * RPC * PADW,
                                    [[RPP * PADW, 128], [1, RPC * PADW]]),
                        in_=pad[:],
                    )

            def node_stage1():
                for t in range(NT):
                    ps = pp.tile([128, 18], f32, tag="ps_n")
                    for k in range(4):
                        xk = pool.tile([128, 128], f32, tag="xk")
                        nc.sync.dma_start(out=xk[:], in_=xt[t, k])
                        nc.tensor.matmul(
                            out=ps[:], lhsT=xk[:], rhs=wf1_sb[:, k * 18:(k + 1) * 18],
                            start=(k == 0), stop=(k == 3),
                        )
                    nc.scalar.copy(out=stage[:, t * ROWW:(t + 1) * ROWW], in_=ps[:, 0:17])
                    nc.vector.tensor_copy(out=ad1[:, t:t + 1], in_=ps[:, 17:18])

            def edge_stage(tabp, adcol, dst_sb, bias_row, meta_off):
                # returns per-tile output written into dst_sb cols (after relu)
                gbuf = {}
                for (g, s, W, off16) in call_meta:
                    gt_ = pool.tile([128, GT * W * ROWW], f32, tag="g%d" % s)
                    for tl0 in range(0, GT, 2):
                        ntl = min(2, GT - tl0)
                        nslots = ntl * 128 * W
                        it = pool.tile([128, GT * 128 * W // 16], i16,
                                       tag="it%d" % (s % 2))
                        nc.sync.dma_start(
                            out=it[:, :nslots // 16],
                            in_=idx_all[:, off16 + tl0 * 8 * W:
                                        off16 + tl0 * 8 * W + nslots // 16],
                        )
                        nc.gpsimd.dma_gather(
                            _ap(gt_[:], [[1, ntl * W], [1, ROWW]],
                                off=tl0 * W * ROWW).rearrange("p (j) w -> p j w")
                            if False else
                            bass.AP(gt_[:].tensor, gt_[:].offset + tl0 * W * ROWW,
                                    [list(gt_[:].ap[0]), [ROWW, ntl * W], [1, ROWW]]),
                            tabp[s * SLAB:min((s + 1) * SLAB, TROWS), :ROWW],
                            it[:, :nslots // 16],
                            nslots,
                            nslots,
                            ROWW,
                            elem_step=PADW,
                            single_packet=False,
                            queue_num=s % NSQ,
                        )
                    gbuf[s] = (gt_, W)
                    if s == NSLAB - 1:
                        for tl in range(GT):
                            t = g * GT + tl
                            Wt = sum(gbuf[ss][1] for ss in range(NSLAB))
                            L = pool.tile([128, Wt], f32, tag="L")
                            wv = pool.tile([128, Wt], f32, tag="wv")
                            wh = pool.tile([128, Wt * 16], f32, tag="wh")
                            off = 0
                            for ss in range(NSLAB):
                                gts, Ws = gbuf[ss]
                                nc.scalar.activation(
                                    out=L[:, off:off + Ws],
                                    in_=_ap(gts[:], [[ROWW, Ws]],
                                            off=tl * Ws * ROWW + 16),
                                    func=mybir.ActivationFunctionType.Identity,
                                    bias=adcol[:, t:t + 1],
                                )
                                off += Ws
                            nc.vector.scalar_tensor_tensor(
                                out=L[:], in0=L[:], scalar=NEG, in1=L[:],
                                op0=mybir.AluOpType.mult,
                                op1=mybir.AluOpType.max,
                            )
                            nc.vector.tensor_scalar_min(L[:], L[:], 60.0)
                            den = pool.tile([128, 1], f32, tag="den")
                            nc.scalar.activation(
                                out=wv[:], in_=L[:],
                                func=mybir.ActivationFunctionType.Exp,
                                accum_out=den[:],
                            )
                            off = 0
                            for ss in range(NSLAB):
                                gts, Ws = gbuf[ss]
                                nc.vector.tensor_tensor(
                                    out=_ap(wh[:], [[16, Ws], [1, 16]], off=off * 16),
                                    in0=_ap(gts[:], [[ROWW, Ws], [1, 16]],
                                            off=tl * Ws * ROWW),
                                    in1=_ap(wv[:], [[1, Ws], [0, 16]], off=off),
                                    op=mybir.AluOpType.mult,
                                )
                                off += Ws
                            num = pool.tile([128, 16], f32, tag="num")
                            nc.vector.tensor_reduce(
                                out=num[:],
                                in_=_ap(wh[:], [[1, 16], [16, Wt]]),
                                axis=mybir.AxisListType.X,
                                op=mybir.AluOpType.add,
                            )
                            rden = pool.tile([128, 1], f32, tag="rden")
                            nc.vector.tensor_scalar_add(den[:], den[:], 1e-30)
                            nc.vector.reciprocal(rden[:], den[:])
                            ot = pool.tile([128, 16], f32, tag="ot")
                            nc.vector.scalar_tensor_tensor(
                                out=ot[:], in0=num[:], scalar=rden[:],
                                in1=bias_row,
                                op0=mybir.AluOpType.mult,
                                op1=mybir.AluOpType.add,
                            )
                            nc.scalar.activation(
                                out=dst_sb[:, t * 16:(t + 1) * 16], in_=ot[:],
                                func=mybir.ActivationFunctionType.Relu,
                            )

            def node_stage2(src_sb, wmat, wcols, dst_stage, adcol):
                for t in range(NT):
                    tp = pp.tile([16, 128], f32, tag="ps_t")
                    nc.tensor.transpose(
                        out=tp[:], in_=src_sb[:, t * 16:(t + 1) * 16], identity=ident[:]
                    )
                    o1t = pool.tile([16, 128], f32, tag="o1t")
                    nc.vector.tensor_copy(out=o1t[:], in_=tp[:])
                    ps = pp.tile([128, wcols], f32, tag="ps_m")
                    nc.tensor.matmul(out=ps[:], lhsT=o1t[:], rhs=wmat[:, :wcols],
                                     start=True, stop=True)
                    if dst_stage is not None:
                        nc.scalar.copy(
                            out=dst_stage[:, t * ROWW:(t + 1) * ROWW], in_=ps[:, 0:17]
                        )
                        nc.vector.tensor_copy(out=adcol[:, t:t + 1], in_=ps[:, 17:18])
                    else:
                        # head: ps = logits pre-bias [128,16]
                        lg = pool.tile([128, 16], f32, tag="lg")
                        nc.vector.tensor_tensor(
                            out=lg[:], in0=ps[:, 0:16],
                            in1=bfull[:, 32:48],
                            op=mybir.AluOpType.add,
                        )
                        mx = pool.tile([128, 1], f32, tag="mx")
                        nc.vector.tensor_reduce(
                            out=mx[:], in_=lg[:], axis=mybir.AxisListType.X,
                            op=mybir.AluOpType.max,
                        )
                        nm = pool.tile([128, 1], f32, tag="nm")
                        nc.vector.tensor_scalar_mul(nm[:], mx[:], -1.0)
                        ex = pool.tile([128, 16], f32, tag="ex")
                        rs = pool.tile([128, 1], f32, tag="rs")
                        nc.scalar.activation(
                            out=ex[:], in_=lg[:],
                            func=mybir.ActivationFunctionType.Exp,
                            bias=nm[:], accum_out=rs[:],
                        )
                        rr = pool.tile([128, 1], f32, tag="rr")
                        nc.vector.reciprocal(rr[:], rs[:])
                        nc.vector.tensor_scalar_mul(
                            _ap(o3[:], [[1, 16]], off=t * 16), ex[:], rr[:]
                        )

            # ---- pipeline ----
            node_stage1()
            write_slice(slice1)
            if STAGES >= 2:
                allgather(slice1, tab1c)
            if STAGES >= 3:
                expand(tab1c, tab1p)
            if STAGES >= 4:
                edge_stage(tab1p, ad1, h2in, bfull[:, 0:16], 0)
            if STAGES >= 5:
                node_stage2(h2in, wf2_sb, 18, stage, ad2)
                write_slice(slice2)
                allgather(slice2, tab2c)
                expand(tab2c, tab2p)
                edge_stage(tab2p, ad2, o2in, bfull[:, 16:32], 0)
                node_stage2(o2in, wo_sb, 16, None, None)
            if STAGES < 5:
                nc.vector.memset(o3[:], 0.5)
            nc.sync.dma_start(
                out=bass.AP(out_d[:].tensor, 0, [[NT * 16, 128], [1, NT * 16]]),
                in_=o3[:],
            )
    nc.compile()
    return nc


_prog_cache = {}
LAST_RUN_S = None


def kernel(**inputs):
    _patch_dma_gather()
    x = np.asarray(inputs["x"], dtype=np.float32)
    edge_index = np.asarray(inputs["edge_index"])
    W1 = np.asarray(inputs["W1"], dtype=np.float32)
    a_src1 = np.asarray(inputs["a_src1"], dtype=np.float32)
    a_dst1 = np.asarray(inputs["a_dst1"], dtype=np.float32)
    b1 = np.asarray(inputs["b1"], dtype=np.float32)
    W2 = np.asarray(inputs["W2"], dtype=np.float32)
    a_src2 = np.asarray(inputs["a_src2"], dtype=np.float32)
    a_dst2 = np.asarray(inputs["a_dst2"], dtype=np.float32)
    b2 = np.asarray(inputs["b2"], dtype=np.float32)
    Wout = np.asarray(inputs["Wout"], dtype=np.float32)
    bout = np.asarray(inputs["bout"], dtype=np.float32)

    prep, _ = _host_prep(x, edge_index)
    wf1 = np.concatenate(
        [W1, (W1 @ a_src1)[:, None], (W1 @ a_dst1)[:, None]], axis=1
    ).astype(np.float32)                                  # [512, 18]
    wf1_t = np.ascontiguousarray(wf1.reshape(4, 128, 18))
    wf2 = np.concatenate(
        [W2, (W2 @ a_src2)[:, None], (W2 @ a_dst2)[:, None]], axis=1
    ).astype(np.float32)                                  # [16, 18]
    bvec = np.concatenate([b1, b2, bout]).astype(np.float32)[None, :]

    # program structure depends on call widths (same for all cores? NO - per core).
    # Build per-core programs is not possible in SPMD; use core 0's meta? Widths
    # differ per core -> pad to the max across cores per (g, s).
    metas = [p["call_meta"] for p in prep]
    merged = []
    for i in range(len(metas[0])):
        g, s, _, _ = metas[0][i]
        W = max(m[i][2] for m in metas)
        merged.append((g, s, W, 0))
    # rebuild idx arrays at merged widths
    off16 = 0
    final_meta = []
    for (g, s, W, _) in merged:
        final_meta.append((g, s, W, off16))
        off16 += GT * 128 * W // 16
    idx_cols = off16

    for c in range(NC):
        idx_new = np.empty((128, idx_cols), dtype=np.int16)
        for i, (g, s, W, off) in enumerate(final_meta):
            _, _, Wc, offc = prep[c]["call_meta"][i]
            s16c = GT * 128 * Wc // 16
            s16 = GT * 128 * W // 16
            chunk = prep[c]["idx_all"][:, offc:offc + s16c]
            # re-pad from width Wc to W: flat k = ((tl*W + j)*128 + p)
            flat_c = np.empty(GT * 128 * Wc, dtype=np.int16)
            wr = chunk[:16, :]                       # [16, s16c]
            flat_c = wr.T.reshape(-1)                # k = s*16+p order
            a3 = flat_c.reshape(GT, Wc, 128)
            a3n = np.full((GT, W, 128), SENT[s] - s * SLAB, dtype=np.int16)
            a3n[:, :Wc, :] = a3
            flat_n = a3n.reshape(-1)
            wrapped = flat_n.reshape(s16, 16).T
            idx_new[:, off:off + s16] = np.tile(wrapped, (8, 1))
        prep[c]["idx_all"] = idx_new

    key = (STAGES,) + tuple((m[0], m[1], m[2]) for m in final_meta)
    if key not in _prog_cache:
        _prog_cache[key] = _build_program(final_meta, idx_cols)
    nc = _prog_cache[key]

    in_maps = []
    for c in range(NC):
        in_maps.append({
            "x_tiled": prep[c]["x_tiled"],
            "idx_all": prep[c]["idx_all"],
            "wf1": wf1_t, "wf2": wf2, "wo": Wout.astype(np.float32),
            "bvec": bvec, "ident": np.eye(128, dtype=np.float32),
        })
    import time as _time
    _t0 = _time.time()
    res = run_bass_kernel_spmd(nc, in_maps, list(range(NC)))
    global LAST_RUN_S
    LAST_RUN_S = _time.time() - _t0

    out = np.empty((N, NCLS), dtype=np.float32)
    q = np.arange(NPOS)
    r_of_q = (q % 128) * NT + q // 128
    for c in range(NC):
        ob = res.results[c]["out"]          # [NPOS, 16] rows r = p*NT+t
        byq = ob[r_of_q]                    # position-ordered
        out[c * NLOC + prep[c]["perm"]] = byq[:NLOC]
    return out
